# revision 1
# baseline (speedup 1.0000x reference)
"""Trainium2 Bass kernel for nn_AttnBlock3d (BatchNorm3d + single-head
self-attention over N=4096 voxels + residual), distributed over 8 NeuronCores.

Sharding: data-parallel over batch (2) x query-quarters (4). Each core
receives its batch's activations (xb), the other batch (xo, stats only),
its query slice (xq), and the weights; it returns its (C, 1024) output
slice. Host assembles the full (B, C, D, H, W) output.

Math notes:
 - BatchNorm folds to hn = x*a + d with a = gamma*rsqrt(var+eps),
   d = beta - mean*a; rsqrt computed as exp(-0.5*ln(var+eps)) so only the
   natural_log_exp ACT table set is needed (shared with softmax exp).
 - Softmax without max-subtraction (scores are O(1) std; fp32 exp safe),
   with a deferred 1/rowsum: the attention+output chain is linear in the
   unnormalized A, so out = inp + r .* (Wo @ (V @ A^T)) + bo'.
 - v-bias folds out: softmax rows sum to 1 -> bo' = bo + Wo@bv.
 - Scores computed transposed (S^T[j,i] = k^T q) so exp'd tiles feed the
   PV and row-sum (ones-vector) matmuls directly as the moving operand.

Scheduling notes:
 - k / v^T production is interleaved into the first attention chunk's
   j-loop (two j-groups ahead) so scores start as soon as the first k
   chunk exists instead of after the whole projection phase.
 - The PV/rowsum matmuls for tile jt are emitted after the scores matmul
   of tile jt+1 (lag-1 software pipeline) so the in-order PE never stalls
   waiting for the exp of the tile it just produced.
"""

import math

import numpy as np

B = 2
C = 128
D = H = W = 16
N_ = 4096
NI = 1024  # queries per core
IC = 512   # i-chunk = one fp32 PSUM bank; fp32 moving-operand max
JT = 128   # j (key) tile = partition dim
EPS = 1e-5
N_CORES = 8

# matmul precision mode: "f32" (exact, slow), "f32r" (fp32 storage,
# relaxed-precision PE mode, full speed at N>=256), "bf16"
MM_MODE = "bf16"

_BUILD_CACHE = {}


def _build(mm_mode, repeat=1):
    from contextlib import ExitStack

    import concourse.bass as bass
    import concourse.mybir as mybir
    import concourse.tile as tile
    from concourse import bacc
    from concourse.bass import ds, ts

    dt = mybir.dt
    f32 = dt.float32
    # f32r: 4-byte reduced-precision matmul format; producers write it rounded
    store_dt = {"bf16": dt.bfloat16, "f32r": dt.float32r, "f32": f32}[mm_mode]
    in_dt = dt.bfloat16 if mm_mode == "bf16" else f32  # xb/xo ship dtype
    Alu = mybir.AluOpType
    Act = mybir.ActivationFunctionType

    def mm(ap):
        return ap

    nc = bacc.Bacc(
        "TRN2", target_bir_lowering=False, debug=False, num_devices=N_CORES
    )

    xb = nc.dram_tensor("xb", (C, N_), in_dt, kind="ExternalInput").ap()
    xo = nc.dram_tensor("xo", (C, N_), in_dt, kind="ExternalInput").ap()
    xq = nc.dram_tensor("xq", (C, NI), f32, kind="ExternalInput").ap()
    # wall = [Wq | Wk | Wv | Wo | I] along columns; vecs = [bq bk bv bo gamma beta]
    wall = nc.dram_tensor("wall", (C, 5 * C), f32, kind="ExternalInput").ap()
    vecs = nc.dram_tensor("vecs", (C, 6), f32, kind="ExternalInput").ap()
    out = nc.dram_tensor("out", (C, NI), f32, kind="ExternalOutput").ap()

    scale = 1.0 / math.sqrt(C)
    NJG = 8          # number of 512-wide j groups
    JG = N_ // NJG   # 512

    with tile.TileContext(nc) as tc, ExitStack() as ctx:
        persist = ctx.enter_context(tc.tile_pool(name="persist", bufs=1))
        small = ctx.enter_context(tc.tile_pool(name="small", bufs=1))
        work = ctx.enter_context(tc.tile_pool(name="work", bufs=3))
        apool = ctx.enter_context(tc.tile_pool(name="apool", bufs=4))
        # PSUM (8 banks): s 2x2-bank pairs = 4 + h 1 + sum 1 + v 2
        pss = ctx.enter_context(tc.tile_pool(name="pss", bufs=2, space="PSUM"))
        psh = ctx.enter_context(tc.tile_pool(name="psh", bufs=1, space="PSUM"))
        pssum = ctx.enter_context(tc.tile_pool(name="pssum", bufs=1, space="PSUM"))
        psv = ctx.enter_context(tc.tile_pool(name="psv", bufs=2, space="PSUM"))

        # body emitted `repeat` times (>1 only for timing NEFFs: the shared
        # bufs=1 persist tiles serialize reps, so the slope is per-rep latency)
        for _rep in range(repeat):
            _body_once(
                nc, tc, persist, small, work, apool, pss, psh, pssum, psv,
                xb, xo, xq, wall, vecs, out, mm, store_dt, in_dt, f32,
                Alu, Act, scale, ts, ds, bass,
            )

    nc.compile()
    return nc


def _body_once(nc, tc, persist, small, work, apool, pss, psh, pssum, psv,
               xb, xo, xq, wall, vecs, out, mm, store_dt, in_dt, f32,
               Alu, Act, scale, ts, ds, bass):
    NJG = 8          # number of 512-wide j groups
    JG = N_ // NJG   # 512
    if True:  # keep indentation of the original body
        # ---- input DMAs (issued in program order; weights early, xo last) ----
        xq_sb = persist.tile([C, NI], f32, tag="xq", name="xq_sb")
        nc.sync.dma_start(out=xq_sb, in_=xq)
        vecs_sb = small.tile([C, 6], f32, tag="vecs", name="vecs_sb")
        nc.sync.dma_start(out=vecs_sb, in_=vecs)
        wall_sb = small.tile([C, 5 * C], f32, tag="wall", name="wall_sb")
        nc.sync.dma_start(out=wall_sb, in_=wall)
        xb_sb = persist.tile([C, N_], in_dt, tag="xb", name="xb_sb")
        for h2 in range(2):
            nc.sync.dma_start(
                out=xb_sb[:, ts(h2, 2048)], in_=xb[:, ts(h2, 2048)]
            )
        xo_sb = persist.tile([C, N_], in_dt, tag="xo", name="xo_sb")
        for h2 in range(2):
            nc.sync.dma_start(
                out=xo_sb[:, ts(h2, 2048)], in_=xo[:, ts(h2, 2048)]
            )
        bq_sb = vecs_sb[:, 0:1]
        bk_sb = vecs_sb[:, 1:2]
        bv_sb = vecs_sb[:, 2:3]
        bo_sb = vecs_sb[:, 3:4]
        gamma_sb = vecs_sb[:, 4:5]
        beta_sb = vecs_sb[:, 5:6]

        # ---- constants ----
        ident = wall_sb[:, ts(4, C)]  # identity shipped with the weights
        ones_row = small.tile([1, C], f32, tag="ones_row", name="ones_row")
        nc.vector.memset(ones_row, 1.0)
        ones_f32 = small.tile([C, 1], f32, tag="ones_f32", name="ones_f32")
        nc.vector.memset(ones_f32, 1.0)
        ones_col = small.tile([C, 1], store_dt, tag="ones_col", name="ones_col")
        nc.vector.tensor_copy(out=ones_col, in_=ones_f32)
        eps_sb = small.tile([C, 1], f32, tag="eps", name="eps_sb")
        nc.vector.memset(eps_sb, EPS)
        zero_sb = small.tile([C, 1], f32, tag="zero", name="zero_sb")
        nc.vector.memset(zero_sb, 0.0)
        # dummy exp: forces the ACT table load at t~0, hidden under the DMAs
        scratch1 = small.tile([C, 1], f32, tag="scratch1", name="scratch1")
        nc.scalar.activation(scratch1, zero_sb, Act.Exp, bias=zero_sb, scale=1.0)

        # ---- weights: transpose on PE -> (c,o) in store_dt ----
        wT = {}
        wTo32 = None
        for i, wname in enumerate(("q", "k", "v", "o")):
            ps_t = psv.tile([C, C], f32, tag="v", name=f"psT_{wname}")
            nc.tensor.transpose(ps_t, wall_sb[:, ts(i, C)], ident)
            wt = small.tile([C, C], store_dt, tag=f"wT_{wname}", name=f"wT_{wname}")
            nc.vector.tensor_copy(out=wt, in_=ps_t)
            wT[wname] = wt
            if wname == "o":
                wTo32 = small.tile([C, C], f32, tag="wTo32", name="wTo32")
                nc.scalar.copy(out=wTo32, in_=ps_t)

        # ---- batchnorm stats over xb & xo (DVE bn_stats) ----
        st = small.tile([C, 16, 6], f32, tag="st", name="st")
        for c8 in range(8):
            nc.vector.bn_stats(
                out=st[:, c8, :], in_=xb_sb[:, ds(c8 * 512, 512)]
            )
        for c8 in range(8):
            nc.vector.bn_stats(
                out=st[:, 8 + c8, :], in_=xo_sb[:, ds(c8 * 512, 512)]
            )
        mv = small.tile([C, 2], f32, tag="mv", name="mv")
        nc.vector.bn_aggr(out=mv, in_=st)
        mean = mv[:, 0:1]
        var = mv[:, 1:2]
        # invstd = exp(-0.5*ln(var+eps))
        lnv = small.tile([C, 1], f32, tag="lnv", name="lnv")
        nc.scalar.activation(lnv, var, Act.Ln, bias=eps_sb, scale=1.0)
        invstd = small.tile([C, 1], f32, tag="invstd", name="invstd")
        nc.scalar.activation(invstd, lnv, Act.Exp, bias=zero_sb, scale=-0.5)
        a_sc = small.tile([C, 1], f32, tag="a_sc", name="a_sc")
        nc.vector.tensor_mul(a_sc, invstd, gamma_sb)
        ma = small.tile([C, 1], f32, tag="ma", name="ma")
        nc.vector.tensor_mul(ma, mean, a_sc)
        d_sc = small.tile([C, 1], f32, tag="d_sc", name="d_sc")
        nc.vector.tensor_sub(d_sc, beta_sb, ma)

        # ---- normalize on ACT (idle here): hn = x*a + d (hnq gates scores) ----
        hnq = persist.tile([C, NI], store_dt, tag="hnq", name="hnq")
        nc.scalar.activation(
            out=hnq, in_=xq_sb, func=Act.Identity, bias=d_sc, scale=a_sc
        )
        hnb = persist.tile([C, N_], store_dt, tag="hnb", name="hnb")
        for c4 in range(4):
            nc.scalar.activation(
                out=hnb[:, ts(c4, 1024)], in_=xb_sb[:, ts(c4, 1024)],
                func=Act.Identity, bias=d_sc, scale=a_sc,
            )

        # ---- q[o,i] with +bq, pre-scaled by 1/sqrt(C) ----
        q_sb = persist.tile([C, NI], store_dt, tag="q", name="q_sb")
        for i2 in range(NI // 512):
            ps_q = psv.tile([C, 512], f32, tag="v", name="ps_q")
            nc.tensor.matmul(
                ps_q, mm(wT["q"]), mm(hnq[:, ts(i2, 512)]), start=True, stop=True
            )
            nc.vector.tensor_scalar(
                out=q_sb[:, ts(i2, 512)], in0=ps_q, scalar1=bq_sb, scalar2=scale,
                op0=Alu.add, op1=Alu.mult,
            )

        # ---- bo' = bo + Wo @ bv ----
        ps_bo = psv.tile([C, 1], f32, tag="v", name="ps_bo")
        nc.tensor.matmul(ps_bo, wTo32, bv_sb, start=True, stop=True)
        bo_col = small.tile([C, 1], f32, tag="bo_col", name="bo_col")
        nc.vector.tensor_add(bo_col, ps_bo, bo_sb)

        # ---- k / v^T production, interleaved into chunk 0 below ----
        k_sb = persist.tile([C, N_], store_dt, tag="k", name="k_sb")
        vT = persist.tile([C, N_], store_dt, tag="vT", name="vT")

        def make_k(g):
            ps_k = psv.tile([C, JG], f32, tag="v", name="ps_k")
            nc.tensor.matmul(
                ps_k, mm(wT["k"]), mm(hnb[:, ts(g, JG)]), start=True, stop=True
            )
            nc.vector.tensor_scalar(
                out=k_sb[:, ts(g, JG)], in0=ps_k, scalar1=bk_sb, scalar2=None,
                op0=Alu.add,
            )

        def make_vT(g):
            # 4 transposed-v tiles into one psum, one copy out
            ps_v = psv.tile([C, JG], f32, tag="v", name="ps_v")
            for t in range(4):
                jt = 4 * g + t
                nc.tensor.matmul(
                    ps_v[:, ts(t, JT)], mm(hnb[:, ts(jt, JT)]), mm(wT["v"]),
                    start=True, stop=True,
                )
            nc.vector.tensor_copy(out=vT[:, ts(g, JG)], in_=ps_v)

        # ---- attention ----
        NJT = N_ // JT
        for icx in range(NI // IC):
            isl = ds(icx * IC, IC)
            h_ps = psh.tile([C, IC], f32, tag="h", name="h_ps")
            sum_ps = pssum.tile([1, IC], f32, tag="sum", name="sum_ps")
            aTs = {}
            if icx == 0:
                make_k(0)
                make_k(1)
                make_vT(0)
                make_vT(1)

            def attend(jt):
                # paired j-tiles: two scores matmuls into one 2-bank psum,
                # a single exp over both (halves ACT per-op overhead)
                s_ps = pss.tile([C, 2 * IC], f32, tag="s", name="s_ps")
                for t in range(2):
                    nc.tensor.matmul(
                        s_ps[:, ts(t, IC)], mm(k_sb[:, ts(jt + t, JT)]),
                        mm(q_sb[:, isl]), start=True, stop=True,
                    )
                aT = apool.tile([C, 2 * IC], store_dt, tag="aT", name="aT")
                nc.scalar.activation(aT, s_ps, Act.Exp, bias=zero_sb, scale=1.0)
                aTs[jt] = aT
                aTs[jt + 1] = aT[:, ts(1, IC)]
                aTs[jt] = aT[:, ts(0, IC)]

            def accum(jt):
                first, last = jt == 0, jt == NJT - 1
                aT = aTs.pop(jt)
                nc.tensor.matmul(
                    h_ps, mm(vT[:, ts(jt, JT)]), mm(aT), start=first, stop=last
                )
                nc.tensor.matmul(
                    sum_ps, mm(ones_col), mm(aT), start=first, stop=last
                )

            # emission: attend pairs (jt, jt+1); accum lags by one pair
            for jp in range(NJT // 2):
                jt = 2 * jp
                if icx == 0 and jt % 4 == 0 and jt // 4 + 2 < NJG:
                    make_k(jt // 4 + 2)
                    make_vT(jt // 4 + 2)
                attend(jt)
                if jp > 0:
                    accum(jt - 2)
                    accum(jt - 1)
            accum(NJT - 2)
            accum(NJT - 1)

            r_row = work.tile([1, IC], f32, tag="r", name="r_row")
            nc.vector.reciprocal(out=r_row, in_=sum_ps)
            # broadcast r down partitions via rank-1 matmul (exact in fp32)
            rb_ps = psv.tile([C, IC], f32, tag="v", name="rb_ps")
            nc.tensor.matmul(rb_ps, ones_row, r_row, start=True, stop=True)
            rb_sb = work.tile([C, IC], f32, tag="rb", name="rb_sb")
            nc.vector.tensor_copy(out=rb_sb, in_=rb_ps)
            h_sb = work.tile([C, IC], store_dt, tag="h_sb", name="h_sb")
            nc.vector.tensor_copy(out=h_sb, in_=h_ps)
            h2_ps = psv.tile([C, IC], f32, tag="v", name="h2_ps")
            nc.tensor.matmul(h2_ps, mm(wT["o"]), mm(h_sb), start=True, stop=True)
            t2 = work.tile([C, IC], f32, tag="t2", name="t2")
            nc.vector.tensor_mul(t2, h2_ps, rb_sb)
            o_sb = work.tile([C, IC], f32, tag="o_sb", name="o_sb")
            nc.vector.scalar_tensor_tensor(
                out=o_sb, in0=t2, scalar=bo_col, in1=xq_sb[:, isl],
                op0=Alu.add, op1=Alu.add,
            )
            nc.sync.dma_start(out=out[:, isl], in_=o_sb)

    nc.compile()
    return nc


def _get_nc(mm_mode=MM_MODE):
    if mm_mode not in _BUILD_CACHE:
        _BUILD_CACHE[mm_mode] = _build(mm_mode)
    return _BUILD_CACHE[mm_mode]


def make_in_maps(inputs, mm_mode=MM_MODE):
    import ml_dtypes

    x = np.ascontiguousarray(
        np.asarray(inputs["inp"], dtype=np.float32).reshape(B, C, N_)
    )
    x_in = x.astype(ml_dtypes.bfloat16) if mm_mode == "bf16" else x
    wall = np.ascontiguousarray(np.concatenate(
        [np.asarray(inputs[k], np.float32) for k in ("Wq", "Wk", "Wv", "Wo")]
        + [np.eye(C, dtype=np.float32)],
        axis=1,
    ))
    vecs = np.ascontiguousarray(np.stack(
        [np.asarray(inputs[k], np.float32).reshape(C)
         for k in ("bq", "bk", "bv", "bo", "gamma", "beta")],
        axis=1,
    ))

    in_maps = []
    for core in range(N_CORES):
        b = core // 4
        q0 = (core % 4) * NI
        in_maps.append({
            "xb": np.ascontiguousarray(x_in[b]),
            "xo": np.ascontiguousarray(x_in[1 - b]),
            "xq": np.ascontiguousarray(x[b][:, q0:q0 + NI]),
            "wall": wall,
            "vecs": vecs,
        })
    return in_maps


def assemble(results):
    out = np.empty((B, C, N_), dtype=np.float32)
    for core in range(N_CORES):
        b = core // 4
        q0 = (core % 4) * NI
        out[b][:, q0:q0 + NI] = results[core]["out"]
    return out.reshape(B, C, D, H, W)


def run(inputs, mm_mode=MM_MODE, **run_kwargs):
    """Run and return (full_output, BassKernelResults)."""
    from concourse.bass_utils import run_bass_kernel_spmd

    nc = _get_nc(mm_mode)
    in_maps = make_in_maps(inputs, mm_mode)
    res = run_bass_kernel_spmd(
        nc, in_maps, core_ids=list(range(N_CORES)), **run_kwargs
    )
    return assemble(res.results), res


def kernel(**inputs):
    out, _ = run(inputs)
    return out



# revision 16
# speedup vs baseline: 1.4141x; 1.4141x over previous
"""Trainium2 Bass kernel for nn_AttnBlok3d (BatchNorm3d + single-head
self-attention over N=4096 voxels + residual), distributed over 8 NeuronCores.

Sharding: data-parallel over batch (2) x query-quarters (4). Each core
receives its batch's activations (xb), the other batch (xo, stats only),
its query slice (xq, fp32 for the residual), and the weights; it returns
its (C, 1024) output slice. Host assembles the full (B, C, D, H, W) output.

Math notes (v2):
 - BatchNorm folds into the projection weights: with a = gamma*rsqrt(var+eps)
   and d = beta - mean*a, q/k/v = W(a.*x) + (W d + b).  The per-channel a is
   multiplied into the (c,o)-transposed weights (per-partition DVE scalar),
   so projections read the raw bf16 x directly -- no normalized copy of x is
   ever materialized.
 - The k-bias (Wk d + bk) shifts every score of a query by a constant; with
   the deferred softmax normalization below it cancels exactly, so k has NO
   bias at all.
 - Wo is folded into V: ovT = x^T (a.*W_ov^T) with W_ov = Wo@Wv, so the
   attention PV matmul directly produces Wo @ (V A).  The v-bias term
   collapses through softmax rows into bo'' = bo + W_ov d + Wo bv, applied
   with the residual.
 - Softmax without max-subtraction, deferred 1/rowsum (out = r .* (OV A)),
   computed as r = exp(-ln(sum)) on ACT (both funcs in the one loaded
   natural_log_exp table set; DVE reciprocal is an 8-cycle/elem iterative
   divide and would cost ~4us on a [1,512] row).
 - Batch stats via one-pass sum/sum-of-squares on the otherwise idle GPSIMD
   engine (accum_out), freeing DVE and overlapping the previous rep.
 - PV + rowsum run as fp8e4m3 DoubleRow matmuls: exp emits A directly in
   fp8 (scores pre-shifted by -4 so exp(s-4) fits e4m3; the shift cancels in
   the deferred normalization), and each matmul contracts TWO 128-wide key
   tiles at 2 MACs/cycle -- half the PE streaming plus half the LDWEIGHTS
   swaps of the bf16 form.

Scheduling notes:
 - k / ovT production is interleaved into the first attention chunk's
   j-loop (two 512-groups ahead).
 - The PV/rowsum matmuls for pair jp are emitted after the scores matmul
   of pair jp+1 (lag-1 software pipeline).
 - All cross-rep tiles are double-buffered (bufs=2) so in the repeated
   timing NEFF rep n+1's DMAs and GPSIMD stats overlap rep n's attention.
"""

import math

import numpy as np

B = 2
C = 128
D = H = W = 16
N_ = 4096
NI = 1024  # queries per core
IC = 512   # i-chunk = one fp32 PSUM bank
JT = 128   # j (key) tile = partition dim
EPS = 1e-5
N_CORES = 8
SHIFT = -4.0  # exp(s + SHIFT); cancels in deferred normalization

# "fp8": PV+rowsum as fp8 DoubleRow; "bf16": plain bf16 attention
MM_MODE = "fp8"

_BUILD_CACHE = {}


def _build(mm_mode, repeat=1):
    from contextlib import ExitStack

    import concourse.bass as bass
    import concourse.mybir as mybir
    import concourse.tile as tile
    from concourse import bacc
    from concourse.bass import ds, ts

    dt = mybir.dt
    f32 = dt.float32

    nc = bacc.Bacc(
        "TRN2", target_bir_lowering=False, debug=False, num_devices=N_CORES
    )

    xb = nc.dram_tensor("xb", (C, N_), dt.bfloat16, kind="ExternalInput").ap()
    xo = nc.dram_tensor("xo", (C, N_), dt.bfloat16, kind="ExternalInput").ap()
    xq = nc.dram_tensor("xq", (C, NI), f32, kind="ExternalInput").ap()
    # wall = [Wq | Wk | Wv | Wo | I] along columns; vecs = [bq bk bv bo gamma beta]
    wall = nc.dram_tensor("wall", (C, 5 * C), f32, kind="ExternalInput").ap()
    vecs = nc.dram_tensor("vecs", (C, 6), f32, kind="ExternalInput").ap()
    out = nc.dram_tensor("out", (C, NI), f32, kind="ExternalOutput").ap()
    dbg = None
    dbg = dbga = dbgq = None
    if mm_mode.endswith("dbg"):
        dbg = nc.dram_tensor("dbg", (C, 3 * NI), f32, kind="ExternalOutput").ap()
        dbga = nc.dram_tensor("dbga", (C, 16 * NI), f32, kind="ExternalOutput").ap()
        dbgq = nc.dram_tensor("dbgq", (C, 3 * NI), f32, kind="ExternalOutput").ap()

    with tile.TileContext(nc) as tc, ExitStack() as ctx:
        persist = ctx.enter_context(tc.tile_pool(name="persist", bufs=2))
        small = ctx.enter_context(tc.tile_pool(name="small", bufs=2))
        work = ctx.enter_context(tc.tile_pool(name="work", bufs=3))
        apool = ctx.enter_context(tc.tile_pool(name="apool", bufs=4))
        # PSUM (8 banks): s pairs 2x2 = 4, h2 1, sum 1, v (prod/epilogue) 2
        pss = ctx.enter_context(tc.tile_pool(name="pss", bufs=2, space="PSUM"))
        psh = ctx.enter_context(tc.tile_pool(name="psh", bufs=1, space="PSUM"))
        pssum = ctx.enter_context(tc.tile_pool(name="pssum", bufs=1, space="PSUM"))
        psv = ctx.enter_context(tc.tile_pool(name="psv", bufs=2, space="PSUM"))

        for _rep in range(repeat):
            _body_once(
                nc, mybir, persist, small, work, apool, pss, psh, pssum, psv,
                xb, xo, xq, wall, vecs, out, mm_mode, ts, ds, dbg, dbga, dbgq,
            )

    nc.compile()
    return nc


def _body_once(nc, mybir, persist, small, work, apool, pss, psh, pssum, psv,
               xb, xo, xq, wall, vecs, out, mm_mode, ts, ds, dbg=None,
               dbga=None, dbgq=None):
    dt = mybir.dt
    f32 = dt.float32
    bf16 = dt.bfloat16
    fp8 = dt.float8e4
    Alu = mybir.AluOpType
    Act = mybir.ActivationFunctionType
    use_fp8 = mm_mode.startswith("fp8")
    a_dt = fp8 if use_fp8 else bf16
    scale = 1.0 / math.sqrt(C)
    NJP = N_ // JT // 2   # 16 j-pairs
    NJG = 8               # production groups of 512
    JG = N_ // NJG

    # ---- input DMAs ----
    vecs_sb = small.tile([C, 6], f32, tag="vecs", name="vecs_sb")
    nc.sync.dma_start(out=vecs_sb, in_=vecs)
    wall_sb = small.tile([C, 5 * C], f32, tag="wall", name="wall_sb")
    nc.sync.dma_start(out=wall_sb, in_=wall)
    xb_sb = persist.tile([C, N_], bf16, tag="xb", name="xb_sb")
    for h2 in range(2):
        nc.sync.dma_start(out=xb_sb[:, ts(h2, 2048)], in_=xb[:, ts(h2, 2048)])
    xo_sb = persist.tile([C, N_], bf16, tag="xo", name="xo_sb")
    for h2 in range(2):
        nc.sync.dma_start(out=xo_sb[:, ts(h2, 2048)], in_=xo[:, ts(h2, 2048)])
    xq_sb = persist.tile([C, NI], f32, tag="xq", name="xq_sb")
    nc.sync.dma_start(out=xq_sb, in_=xq)

    bq_sb = vecs_sb[:, 0:1]
    bv_sb = vecs_sb[:, 2:3]
    bo_sb = vecs_sb[:, 3:4]
    gamma_sb = vecs_sb[:, 4:5]
    beta_sb = vecs_sb[:, 5:6]
    ident = wall_sb[:, ts(4, C)]

    # ---- constants ----
    ones_row = small.tile([1, C], f32, tag="ones_row", name="ones_row")
    nc.vector.memset(ones_row, 1.0)
    if use_fp8:
        ones2 = small.tile([C, 2, 16], fp8, tag="ones2", name="ones2")
        nc.vector.memset(ones2, 1.0)
    else:
        ones2 = small.tile([C, 1], bf16, tag="ones2", name="ones2")
        nc.vector.memset(ones2, 1.0)
    z1 = small.tile([C, 1], f32, tag="z1", name="z1")
    nc.vector.memset(z1, 0.0)
    eps_sb = small.tile([C, 1], f32, tag="eps", name="eps_sb")
    nc.vector.memset(eps_sb, EPS)
    shift_sb = small.tile([C, 1], f32, tag="shift", name="shift_sb")
    nc.vector.memset(shift_sb, SHIFT)
    # dummy exp: forces the single natural_log_exp ACT table load at t~0
    scr1 = small.tile([C, 1], f32, tag="scr1", name="scr1")
    nc.scalar.activation(scr1, z1, Act.Exp, bias=z1, scale=1.0)

    # ---- weight transposes (PE) -> fp32 SBUF copies (ACT) ----
    wT32 = {}
    for i, wname in ((3, "o"), (0, "q"), (1, "k")):
        ps_t = psv.tile([C, C], f32, tag="v", name=f"psT_{wname}")
        nc.tensor.transpose(ps_t, wall_sb[:, ts(i, C)], ident)
        wt = small.tile([C, C], f32, tag=f"wT32_{wname}", name=f"wT32_{wname}")
        nc.scalar.copy(out=wt, in_=ps_t)
        wT32[wname] = wt
    # W_ov^T[c,o'] = sum_o Wv[o,c] * Wo^T[o,o']  (contraction over o)
    ps_ov = psv.tile([C, C], f32, tag="v", name="ps_ov")
    nc.tensor.matmul(ps_ov, wall_sb[:, ts(2, C)], wT32["o"], start=True, stop=True)
    wTov32 = small.tile([C, C], f32, tag="wT32_ov", name="wTov32")
    nc.scalar.copy(out=wTov32, in_=ps_ov)

    # ---- batch stats on GPSIMD: sums and sum-of-squares per channel ----
    # out is a stride-0 dummy (no real writes); only accum_out matters.
    dummy = small.tile([C, 1], bf16, tag="dummy", name="dummy")
    sums = small.tile([C, 4], f32, tag="sums", name="sums")
    for idx, xt in ((0, xb_sb), (1, xo_sb)):
        nc.vector.tensor_scalar(
            out=dummy.broadcast_to(xt.shape), in0=xt,
            scalar1=1.0, scalar2=0.0, op0=Alu.mult, op1=Alu.add,
            accum_out=sums[:, idx:idx + 1],
        )
        nc.vector.scalar_tensor_tensor(
            out=dummy.broadcast_to(xt.shape), in0=xt, scalar=1.0, in1=xt,
            op0=Alu.mult, op1=Alu.mult,
            accum_out=sums[:, 2 + idx:3 + idx],
        )
    mean = small.tile([C, 1], f32, tag="mean", name="mean")
    t0 = small.tile([C, 1], f32, tag="t0", name="t0")
    nc.vector.tensor_add(t0, sums[:, 0:1], sums[:, 1:2])
    nc.vector.tensor_scalar(
        out=mean, in0=t0, scalar1=1.0 / (2 * N_), scalar2=None, op0=Alu.mult
    )
    msq = small.tile([C, 1], f32, tag="msq", name="msq")
    nc.vector.tensor_add(t0, sums[:, 2:3], sums[:, 3:4])
    nc.vector.tensor_scalar(
        out=msq, in0=t0, scalar1=1.0 / (2 * N_), scalar2=None, op0=Alu.mult
    )
    var = small.tile([C, 1], f32, tag="var", name="var")
    nc.vector.tensor_mul(t0, mean, mean)
    nc.vector.tensor_sub(var, msq, t0)
    # invstd = exp(-0.5*ln(var+eps)); a = gamma*invstd; d = beta - mean*a
    lnv = small.tile([C, 1], f32, tag="lnv", name="lnv")
    nc.scalar.activation(lnv, var, Act.Ln, bias=eps_sb, scale=1.0)
    invstd = small.tile([C, 1], f32, tag="invstd", name="invstd")
    nc.scalar.activation(invstd, lnv, Act.Exp, bias=z1, scale=-0.5)
    a_sc = small.tile([C, 1], f32, tag="a_sc", name="a_sc")
    nc.vector.tensor_mul(a_sc, invstd, gamma_sb)
    d_sc = small.tile([C, 1], f32, tag="d_sc", name="d_sc")
    nc.vector.tensor_mul(t0, mean, a_sc)
    nc.vector.tensor_sub(d_sc, beta_sb, t0)

    # ---- scaled bf16 weights: wTq'' = wTq*(a*scale), wTk' = wTk*a, wTov' ----
    wTq_s = small.tile([C, C], bf16, tag="wTq_s", name="wTq_s")
    nc.vector.tensor_scalar(
        out=wTq_s, in0=wT32["q"], scalar1=a_sc, scalar2=scale,
        op0=Alu.mult, op1=Alu.mult,
    )
    wTk_s = small.tile([C, C], bf16, tag="wTk_s", name="wTk_s")
    nc.vector.tensor_scalar(
        out=wTk_s, in0=wT32["k"], scalar1=a_sc, scalar2=None, op0=Alu.mult
    )
    wTov_s = small.tile([C, C], bf16, tag="wTov_s", name="wTov_s")
    nc.vector.tensor_scalar(
        out=wTov_s, in0=wTov32, scalar1=a_sc, scalar2=None, op0=Alu.mult
    )

    # ---- bias fixups ----
    # bq'' = scale*(Wq d + bq)
    ps_bq = psv.tile([C, 1], f32, tag="v", name="ps_bq")
    nc.tensor.matmul(ps_bq, wT32["q"], d_sc, start=True, stop=True)
    bq_eff = small.tile([C, 1], f32, tag="bq_eff", name="bq_eff")
    nc.vector.tensor_scalar(
        out=bq_eff, in0=ps_bq, scalar1=bq_sb, scalar2=scale,
        op0=Alu.add, op1=Alu.mult,
    )
    # bo'' = bo + W_ov d + Wo bv
    ps_bo = psv.tile([C, 1], f32, tag="v", name="ps_bo")
    nc.tensor.matmul(ps_bo, wTov32, d_sc, start=True, stop=False)
    nc.tensor.matmul(ps_bo, wT32["o"], bv_sb, start=False, stop=True)
    bo_col = small.tile([C, 1], f32, tag="bo_col", name="bo_col")
    nc.vector.tensor_add(bo_col, ps_bo, bo_sb)
    # resid = xq + bo''  (applied post-normalization; see epilogue)
    resid = persist.tile([C, NI], f32, tag="resid", name="resid")
    nc.vector.tensor_scalar(
        out=resid, in0=xq_sb, scalar1=bo_col, scalar2=None, op0=Alu.add
    )

    # ---- q[o,i] = wTq''^T x + bq''  (q's x comes from the fp32 xq slice,
    # cast to bf16 on-device: the per-core slice offset can't be a slice of
    # xb in a single shared SPMD program) ----
    q_sb = persist.tile([C, NI], bf16, tag="q", name="q_sb")
    qx = persist.tile([C, NI], bf16, tag="qx", name="qx")
    nc.vector.tensor_copy(out=qx, in_=xq_sb)
    for i2 in range(NI // 512):
        ps_q = psv.tile([C, 512], f32, tag="v", name="ps_q")
        nc.tensor.matmul(
            ps_q, wTq_s, qx[:, ts(i2, 512)], start=True, stop=True
        )
        nc.vector.tensor_scalar(
            out=q_sb[:, ts(i2, 512)], in0=ps_q, scalar1=bq_eff, scalar2=None,
            op0=Alu.add,
        )

    if dbgq is not None:
        d_q = persist.tile([C, NI], f32, tag="d_q", name="d_q")
        nc.vector.tensor_copy(out=d_q, in_=q_sb)
        nc.sync.dma_start(out=dbgq[:, 0:NI], in_=d_q)
        d_qx = persist.tile([C, NI], f32, tag="d_qx", name="d_qx")
        nc.vector.tensor_copy(out=d_qx, in_=qx)
        nc.sync.dma_start(out=dbgq[:, NI:2 * NI], in_=d_qx)

    # ---- k / ovT production (interleaved into chunk 0 below) ----
    k_sb = persist.tile([C, N_], bf16, tag="k", name="k_sb")
    ovT = persist.tile([C, N_ // JT, JT], a_dt, tag="ovT", name="ovT")

    def make_k(g):
        ps_k = psv.tile([C, JG], f32, tag="v", name="ps_k")
        nc.tensor.matmul(
            ps_k, wTk_s, xb_sb[:, ts(g, JG)], start=True, stop=True
        )
        nc.vector.tensor_copy(out=k_sb[:, ts(g, JG)], in_=ps_k)

    def make_ov(g):
        ps_v = psv.tile([C, JG], f32, tag="v", name="ps_v")
        for t in range(4):
            jt = 4 * g + t
            nc.tensor.matmul(
                ps_v[:, ts(t, JT)], xb_sb[:, ts(jt, JT)], wTov_s,
                start=True, stop=True,
            )
        nc.vector.tensor_copy(out=ovT[:, 4 * g:4 * g + 4, :], in_=ps_v)

    # ---- attention ----
    for icx in range(NI // IC):
        isl = ds(icx * IC, IC)
        h2_ps = psh.tile([C, IC], f32, tag="h", name="h2_ps")
        sum_ps = pssum.tile([1, IC], f32, tag="sum", name="sum_ps")
        aTs = {}
        if icx == 0:
            make_k(0)
            make_k(1)
            make_ov(0)
            make_ov(1)

        def attend(jp):
            s_ps = pss.tile([C, 2, IC], f32, tag="s", name="s_ps")
            for t in range(2):
                nc.tensor.matmul(
                    s_ps[:, t, :], k_sb[:, ts(2 * jp + t, JT)],
                    q_sb[:, isl], start=True, stop=True,
                )
            aT = apool.tile([C, 2, IC], a_dt, tag="aT", name="aT")
            nc.scalar.activation(
                aT, s_ps, Act.Exp, bias=shift_sb if use_fp8 else z1, scale=1.0
            )
            aTs[jp] = aT
            if dbgq is not None and icx == 0 and jp == 4:
                d_s = work.tile([C, 2 * IC], f32, tag="d_s", name="d_s")
                nc.vector.tensor_copy(out=d_s, in_=s_ps)
                nc.sync.dma_start(out=dbgq[:, 2 * NI:3 * NI], in_=d_s)
            if dbga is not None and icx == 0:
                d_a = work.tile([C, 2 * IC], f32, tag="d_a", name="d_a")
                nc.vector.tensor_copy(out=d_a, in_=aT)
                nc.sync.dma_start(out=dbga[:, ds(jp * 2 * IC, 2 * IC)], in_=d_a)

        def accum(jp):
            first, last = jp == 0, jp == NJP - 1
            aT = aTs.pop(jp)
            if use_fp8:
                rhs = aT  # [K, 2, N]: two j_lo streams, dim1 step%16==0
                nc.tensor.matmul(
                    h2_ps, ovT[:, 2 * jp:2 * jp + 2, :], rhs,
                    start=first, stop=last,
                    perf_mode=mybir.MatmulPerfMode.DoubleRow,
                )
                nc.tensor.matmul(
                    sum_ps, ones2[:, :, 0:1], rhs,
                    start=first, stop=last,
                    perf_mode=mybir.MatmulPerfMode.DoubleRow,
                )
            else:
                for t in range(2):
                    nc.tensor.matmul(
                        h2_ps, ovT[:, 2 * jp + t, :], aT[:, t, :],
                        start=first and t == 0, stop=last and t == 1,
                    )
                    nc.tensor.matmul(
                        sum_ps, ones2, aT[:, t, :],
                        start=first and t == 0, stop=last and t == 1,
                    )

        for jp in range(NJP):
            if icx == 0 and jp % 2 == 0 and jp // 2 + 2 < NJG:
                make_k(jp // 2 + 2)
                make_ov(jp // 2 + 2)
            attend(jp)
            if jp > 0:
                accum(jp - 1)
        accum(NJP - 1)

        # ---- epilogue: r = exp(-ln(sum)); out = r.*h2 + resid ----
        lnr = work.tile([1, IC], f32, tag="lnr", name="lnr")
        nc.scalar.activation(lnr, sum_ps, Act.Ln, bias=z1[0:1, :], scale=1.0)
        r_row = work.tile([1, IC], f32, tag="r", name="r_row")
        nc.scalar.activation(r_row, lnr, Act.Exp, bias=z1[0:1, :], scale=-1.0)
        rb_ps = psv.tile([C, IC], f32, tag="v", name="rb_ps")
        nc.tensor.matmul(rb_ps, ones_row, r_row, start=True, stop=True)
        rb_sb = work.tile([C, IC], f32, tag="rb", name="rb_sb")
        nc.vector.tensor_copy(out=rb_sb, in_=rb_ps)
        t2 = work.tile([C, IC], f32, tag="t2", name="t2")
        nc.vector.tensor_mul(t2, h2_ps, rb_sb)
        o_sb = work.tile([C, IC], f32, tag="o_sb", name="o_sb")
        nc.vector.tensor_add(o_sb, t2, resid[:, isl])
        nc.sync.dma_start(out=out[:, isl], in_=o_sb)
        if dbg is not None:
            d_h2 = work.tile([C, IC], f32, tag="d_h2", name="d_h2")
            nc.vector.tensor_copy(out=d_h2, in_=h2_ps)
            nc.sync.dma_start(out=dbg[:, ds(icx * IC, IC)], in_=d_h2)
            nc.sync.dma_start(out=dbg[:, ds(NI + icx * IC, IC)], in_=rb_sb)
            d_sm = work.tile([1, IC], f32, tag="d_sm", name="d_sm")
            nc.vector.tensor_copy(out=d_sm, in_=sum_ps)
            nc.sync.dma_start(out=dbg[0:1, ds(2 * NI + icx * IC, IC)], in_=d_sm)


def _get_nc(mm_mode=MM_MODE):
    if mm_mode not in _BUILD_CACHE:
        _BUILD_CACHE[mm_mode] = _build(mm_mode)
    return _BUILD_CACHE[mm_mode]


def make_in_maps(inputs, mm_mode=MM_MODE):
    import ml_dtypes

    x = np.ascontiguousarray(
        np.asarray(inputs["inp"], dtype=np.float32).reshape(B, C, N_)
    )
    x_bf = x.astype(ml_dtypes.bfloat16)
    wall = np.ascontiguousarray(np.concatenate(
        [np.asarray(inputs[k], np.float32) for k in ("Wq", "Wk", "Wv", "Wo")]
        + [np.eye(C, dtype=np.float32)],
        axis=1,
    ))
    vecs = np.ascontiguousarray(np.stack(
        [np.asarray(inputs[k], np.float32).reshape(C)
         for k in ("bq", "bk", "bv", "bo", "gamma", "beta")],
        axis=1,
    ))

    in_maps = []
    for core in range(N_CORES):
        b = core // 4
        q0 = (core % 4) * NI
        in_maps.append({
            "xb": np.ascontiguousarray(x_bf[b]),
            "xo": np.ascontiguousarray(x_bf[1 - b]),
            "xq": np.ascontiguousarray(x[b][:, q0:q0 + NI]),
            "wall": wall,
            "vecs": vecs,
        })
    return in_maps


def assemble(results):
    out = np.empty((B, C, N_), dtype=np.float32)
    for core in range(N_CORES):
        b = core // 4
        q0 = (core % 4) * NI
        out[b][:, q0:q0 + NI] = results[core]["out"]
    return out.reshape(B, C, D, H, W)


def run(inputs, mm_mode=MM_MODE, **run_kwargs):
    """Run and return (full_output, BassKernelResults)."""
    from concourse.bass_utils import run_bass_kernel_spmd

    nc = _get_nc(mm_mode)
    in_maps = make_in_maps(inputs, mm_mode)
    res = run_bass_kernel_spmd(
        nc, in_maps, core_ids=list(range(N_CORES)), **run_kwargs
    )
    return assemble(res.results), res


def kernel(**inputs):
    out, _ = run(inputs)
    return out
